# revision 7
# baseline (speedup 1.0000x reference)
"""BlockSparseLocallyConnected forward on 8 Trainium2 NeuronCores.

Window-column shard: core k owns output columns nc in {4k..4k+3}, all 64
batches.  The PE does the real MACs (the DVE tensor_tensor path is capped
at 2x = 34us/core; the PE stream floor is 27.3us/core):

  out[b, nr, nc] = sum_{dr,dc} xpad[b, 16nr+dr, 16nc+dc] * w[nr*32+nc, dr*32+dc]

Contraction (dr, dc) is split into 8 chunks q=(qr, hc) of 128 = (dr_local 8,
c16 16); SBUF partition p = 16*dr_local + c16 holds x rows r = dr_local
(mod 8), cols c = c16 (mod 16) -- window columns start at multiples of 16,
so ONE copy of x serves every (nc, hc) with a pure free-dim offset.  Rows
are stored per partition as [b, m', par, idx] with r = 16*idx + 8*par +
dr_local, so the moving AP for window-row nr_x is contiguous (stride 1).

Per (nc_local j, q): lhsT = weights [128, 32 nr_w] (stationary), rhs = x
[128, (b 16, nr_x 32) = 512] (moving), accumulated over the 8 q-chunks into
PSUM[32j:32j+32, 512] via tile_position=(0, 32j).  The matmul computes all
(nr_w, nr_x) cross terms; only the diagonal nr_w == nr_x is the real
output.  ACT evacuates PSUM -> SBUF adding the per-partition bias, the
full [128, 512] tiles DMA out, and the host gathers the diagonal (free).
"""

import sys

sys.path.insert(0, "/opt/trn_rl_repo")

import numpy as np
import ml_dtypes

# ---- problem constants (hardcoded; kernel.py must be self-contained) ----
B = 64            # batch
H = W = 512
PH = PW = 8
FULL = 528        # padded H/W
NKH = NKW = 32    # window grid
NCORES = 8
NCL = 4           # window-columns per core
FQ = 4            # f-dim chunks (16 batches each)
BFQ = B // FQ     # 16
M = 5             # 16-col blocks per core span (80 cols)

BF16 = ml_dtypes.bfloat16

_CACHE = {}

TRACE = False          # test.py sets True to get exec_time_ns
LAST_RESULTS = None    # BassKernelResults of last run (for test.py)


def _build_program():
    import concourse.bass as bass
    import concourse.bacc as bacc
    import concourse.tile as tile
    from concourse import mybir

    dt_c = mybir.dt.bfloat16
    f32 = mybir.dt.float32

    nc = bacc.Bacc(
        "TRN2", target_bir_lowering=False, debug=False, num_devices=NCORES
    )
    # x: [fq, m, p, bi, par, idx] -- each (fq, m) slab is one contiguous DMA
    xs = nc.dram_tensor("xs", [FQ, M, 128, BFQ, 2, 33], dt_c, kind="ExternalInput")
    # weights: [p, j, qr, hc, nr_w]
    wp = nc.dram_tensor("wp", [128, NCL, 4, 2, 32], dt_c, kind="ExternalInput")
    bp = nc.dram_tensor("bp", [128, 1], f32, kind="ExternalInput")
    out_d = nc.dram_tensor("out", [FQ, NCL, 32, 512], f32, kind="ExternalOutput")

    with tile.TileContext(nc) as tc:
        with (
            tc.tile_pool(name="xpool", bufs=FQ * M) as xpool,
            tc.tile_pool(name="cst", bufs=1) as cst,
            tc.tile_pool(name="psum", bufs=3, space="PSUM") as psum,
            tc.tile_pool(name="opool", bufs=2 * NCL) as opool,
        ):
            # Tiny wake transfers on each issue ring: DMA engines take ~3us
            # to come up after their first descriptor, so poke them at t=0.
            wk_sb = cst.tile([128, 2], dt_c)
            nc.sync.dma_start(out=wk_sb[:], in_=xs[0, 0, :, 0, 0, 0:2])
            nc.scalar.dma_start(out=wk_sb[:], in_=xs[0, 1, :, 0, 0, 0:2])
            nc.gpsimd.dma_start(out=wk_sb[:], in_=xs[0, 2, :, 0, 0, 0:2])

            # PE warmup during the DMA ramp: back-to-back matmuls push HAM
            # toward full clock before the real stream starts.  (wpsum is
            # never read; the memset runs on the otherwise-idle gpsimd.)
            warm = cst.tile([128, 512], dt_c)
            nc.gpsimd.memset(warm[:], 1.0)
            wpsum = psum.tile([128, 512], f32, tag="warm")
            for _ in range(8):
                nc.tensor.matmul(wpsum[:], warm[:, 0:128], warm[:],
                                 start=True, stop=True)

            # Input DMAs spread over three issue rings (each dma_start costs
            # ~0.6us of ring-issue time; one ring would serialize ~13us).
            rings = [nc.sync, nc.scalar, nc.gpsimd]
            w_sb = cst.tile([128, NCL, 4, 2, 32], dt_c)
            nc.sync.dma_start(out=w_sb[:], in_=wp[:])
            b_sb = cst.tile([128, 1], f32)
            nc.gpsimd.dma_start(out=b_sb[:], in_=bp[:])
            x_sb = [[None] * M for _ in range(FQ)]
            n = 0
            for fq in range(FQ):
                for m in range(M):
                    x_sb[fq][m] = xpool.tile(
                        [128, BFQ, 2, 33], dt_c, tag="xb", name=f"xb_{fq}_{m}"
                    )
                    rings[n % 3].dma_start(out=x_sb[fq][m][:], in_=xs[fq, m])
                    n += 1

            # Real stream: per fq, 8 q-chunks x 4 j = 32 matmuls into one
            # PSUM bank.  Ordered by s = j + hc so each matmul only needs
            # x slab m' = j + hc (DMA arrives m-ascending).
            order = []  # (s, j, hc)
            for j in range(NCL):
                order.append((j + 0, j, 0))
                order.append((j + 1, j, 1))
            order.sort()
            for fq in range(FQ):
                ps = psum.tile([128, 512], f32, tag="acc", name=f"acc{fq}")
                seen = [0] * NCL
                for s, j, hc in order:
                    for qr in range(4):
                        xt = x_sb[fq][j + hc][:]
                        rhs = bass.AP(
                            tensor=xt.tensor,
                            offset=xt.offset + 33 * (qr & 1) + (qr >> 1),
                            ap=[
                                list(xt.ap[0]),  # partition
                                [66, BFQ],       # b
                                [1, 32],         # nr_x
                            ],
                        )
                        nc.tensor.matmul(
                            ps[32 * j: 32 * j + 32, :],
                            w_sb[:, j, qr, hc, :],
                            rhs,
                            start=(seen[j] == 0),
                            stop=(seen[j] == 7),
                            tile_position=(0, 32 * j),
                        )
                        seen[j] += 1
                        # evac region j as soon as its 8 chunks are in
                        if seen[j] == 8:
                            ev = opool.tile([32, 512], f32, tag="ev",
                                            name=f"ev{fq}_{j}")
                            nc.scalar.activation(
                                out=ev[:], in_=ps[32 * j: 32 * j + 32, :],
                                func=mybir.ActivationFunctionType.Identity,
                                bias=b_sb[32 * j: 32 * j + 32], scale=1.0,
                            )
                            nc.scalar.dma_start(out=out_d[fq, j], in_=ev[:])
    nc.compile()
    return nc


def _prep_inputs(x, weight, bias):
    """Host-side packing into the transposed (mod-8 row, mod-16 col)
    partition layout; bf16 cast.  Returns per-core in_maps."""
    x = np.asarray(x, dtype=np.float32)
    weight = np.asarray(weight, dtype=np.float32)
    bias = np.asarray(bias, dtype=np.float32)

    xpad = np.zeros((B, FULL, FULL), dtype=np.float32)
    xpad[:, PH:PH + H, PW:PW + W] = x[:, 0]
    xpb = xpad.astype(BF16)

    # r = 16*idx + 8*par + dl
    dl = np.arange(8)[:, None, None]
    par = np.arange(2)[None, :, None]
    idx = np.arange(33)[None, None, :]
    r_map = 16 * idx + 8 * par + dl                      # [8, 2, 33]

    w4 = weight.reshape(32, 32, 32, 32)                  # [nr, nc, dr, dc]
    bv = bias.reshape(32, 32)                            # [nr, nc]

    in_maps = []
    for k in range(NCORES):
        c_map = (16 * (4 * k + np.arange(M))[:, None]
                 + np.arange(16)[None, :])               # [m, c16]
        # gather -> [b, dl, par, idx, m, c16]
        g = xpb[:, r_map.reshape(8, 2, 33, 1, 1),
                c_map.reshape(1, 1, 1, M, 16)]
        # -> [fq, bi, dl, par, idx, m, c16]
        g = g.reshape(FQ, BFQ, 8, 2, 33, M, 16)
        # -> [fq, m, dl, c16, bi, par, idx]
        g = g.transpose(0, 5, 2, 6, 1, 3, 4)
        xs = np.ascontiguousarray(g.reshape(FQ, M, 128, BFQ, 2, 33))

        # weights: [nr, j, qr, dl, hc, c16] -> [dl, c16, j, qr, hc, nr]
        wk = w4[:, 4 * k:4 * k + NCL].reshape(32, NCL, 4, 8, 2, 16)
        wk = wk.transpose(3, 5, 1, 2, 4, 0)
        wpk = np.ascontiguousarray(wk.reshape(128, NCL, 4, 2, 32)).astype(BF16)

        # bias: partition 32j + nr_w -> bias[nr_w, 4k+j]
        bk = np.ascontiguousarray(
            bv[:, 4 * k:4 * k + NCL].T.reshape(128, 1))

        in_maps.append({"xs": xs, "wp": wpk, "bp": bk})
    return in_maps


def kernel(x, weight, bias):
    global LAST_RESULTS
    from concourse.bass_utils import run_bass_kernel_spmd

    if "nc" not in _CACHE:
        _CACHE["nc"] = _build_program()
    nc = _CACHE["nc"]

    in_maps = _prep_inputs(x, weight, bias)
    res = run_bass_kernel_spmd(
        nc, in_maps, core_ids=list(range(NCORES)), trace=TRACE
    )
    LAST_RESULTS = res

    out = np.empty((B, NKH, NKW), dtype=np.float32)
    ar = np.arange(32)
    for k in range(NCORES):
        r5 = res.results[k]["out"].reshape(FQ, NCL, 32, BFQ, 32)
        d = r5[:, :, ar, :, ar]                 # [nr, fq, j, bi]
        d = d.transpose(1, 3, 0, 2)             # [fq, bi, nr, j]
        out[:, :, 4 * k:4 * k + NCL] = d.reshape(B, NKH, NCL)
    return out


# revision 8
# speedup vs baseline: 1.1304x; 1.1304x over previous
"""BlockSparseLocallyConnected forward on 8 Trainium2 NeuronCores.

Window-column shard: core k owns output columns nc in {4k..4k+3}, all 64
batches.  The PE does the real MACs (the DVE tensor_tensor path is capped
at 2x = 34us/core; the PE stream floor is 27.3us/core):

  out[b, nr, nc] = sum_{dr,dc} xpad[b, 16nr+dr, 16nc+dc] * w[nr*32+nc, dr*32+dc]

Contraction (dr, dc) is split into 8 chunks q=(qr, hc) of 128 = (dr_local 8,
c16 16); SBUF partition p = 16*dr_local + c16 holds x rows r = dr_local
(mod 8), cols c = c16 (mod 16) -- window columns start at multiples of 16,
so ONE copy of x serves every (nc, hc) with a pure free-dim offset.  Rows
are stored per partition as [b, m', par, idx] with r = 16*idx + 8*par +
dr_local, so the moving AP for window-row nr_x is contiguous (stride 1).

Per (nc_local j, q): lhsT = weights [128, 32 nr_w] (stationary), rhs = x
[128, (b 16, nr_x 32) = 512] (moving), accumulated over the 8 q-chunks into
PSUM[32j:32j+32, 512] via tile_position=(0, 32j).  The matmul computes all
(nr_w, nr_x) cross terms; only the diagonal nr_w == nr_x is the real
output.  ACT evacuates PSUM -> SBUF adding the per-partition bias, the
full [128, 512] tiles DMA out, and the host gathers the diagonal (free).
"""

import sys

sys.path.insert(0, "/opt/trn_rl_repo")

import numpy as np
import ml_dtypes

# ---- problem constants (hardcoded; kernel.py must be self-contained) ----
B = 64            # batch
H = W = 512
PH = PW = 8
FULL = 528        # padded H/W
NKH = NKW = 32    # window grid
NCORES = 8
NCL = 4           # window-columns per core
FQ = 4            # f-dim chunks (16 batches each)
BFQ = B // FQ     # 16
M = 5             # 16-col blocks per core span (80 cols)

BF16 = ml_dtypes.bfloat16

_CACHE = {}

TRACE = False          # test.py sets True to get exec_time_ns
LAST_RESULTS = None    # BassKernelResults of last run (for test.py)


def _build_program():
    import concourse.bass as bass
    import concourse.bacc as bacc
    import concourse.tile as tile
    from concourse import mybir

    dt_c = mybir.dt.bfloat16
    f32 = mybir.dt.float32

    nc = bacc.Bacc(
        "TRN2", target_bir_lowering=False, debug=False, num_devices=NCORES
    )
    # x: [fq, m, p, bi, par, idx] -- each (fq, m) slab is one contiguous DMA
    xs = nc.dram_tensor("xs", [FQ, M, 128, BFQ, 2, 33], dt_c, kind="ExternalInput")
    # weights: [p, j, qr, hc, nr_w]
    wp = nc.dram_tensor("wp", [128, NCL, 4, 2, 32], dt_c, kind="ExternalInput")
    bp = nc.dram_tensor("bp", [128, 1], f32, kind="ExternalInput")
    out_d = nc.dram_tensor("out", [FQ, NCL, 32, 512], f32, kind="ExternalOutput")

    with tile.TileContext(nc) as tc:
        with (
            tc.tile_pool(name="xpool", bufs=FQ * M) as xpool,
            tc.tile_pool(name="cst", bufs=1) as cst,
            tc.tile_pool(name="psum", bufs=3, space="PSUM") as psum,
            tc.tile_pool(name="opool", bufs=2 * NCL) as opool,
        ):
            # Tiny wake transfers on each issue ring: DMA engines take ~3us
            # to come up after their first descriptor, so poke them at t=0.
            wk_sb = cst.tile([128, 2], dt_c)
            nc.sync.dma_start(out=wk_sb[:], in_=xs[0, 0, :, 0, 0, 0:2])
            nc.scalar.dma_start(out=wk_sb[:], in_=xs[0, 1, :, 0, 0, 0:2])
            nc.gpsimd.dma_start(out=wk_sb[:], in_=xs[0, 2, :, 0, 0, 0:2])

            # PE warmup during the DMA ramp: back-to-back matmuls push HAM
            # toward full clock before the real stream starts.  (wpsum is
            # never read; the memset runs on the otherwise-idle gpsimd.)
            warm = cst.tile([128, 512], dt_c)
            nc.gpsimd.memset(warm[:], 1.0)
            wpsum = psum.tile([128, 512], f32, tag="warm")
            for _ in range(8):
                nc.tensor.matmul(wpsum[:], warm[:, 0:128], warm[:],
                                 start=True, stop=True)

            # All input DMAs on ONE ring (sync), in exact consumption order:
            # competing rings starve the stream (measured: PE gaps every
            # ~8 MMs when x slabs round-robin over rings).
            w_sb = cst.tile([128, NCL, 4, 2, 32], dt_c)
            nc.sync.dma_start(out=w_sb[:], in_=wp[:])
            b_sb = cst.tile([128, 1], f32)
            nc.gpsimd.dma_start(out=b_sb[:], in_=bp[:])
            x_sb = [[None] * M for _ in range(FQ)]
            for fq in range(FQ):
                for m in range(M):
                    x_sb[fq][m] = xpool.tile(
                        [128, BFQ, 2, 33], dt_c, tag="xb", name=f"xb_{fq}_{m}"
                    )
                    nc.sync.dma_start(out=x_sb[fq][m][:], in_=xs[fq, m])

            # Real stream: per fq, 8 q-chunks x 4 j = 32 matmuls into one
            # PSUM bank.  Ordered by s = j + hc so each matmul only needs
            # x slab m' = j + hc (DMA arrives m-ascending).
            order = []  # (s, j, hc)
            for j in range(NCL):
                order.append((j + 0, j, 0))
                order.append((j + 1, j, 1))
            order.sort()
            for fq in range(FQ):
                ps = psum.tile([128, 512], f32, tag="acc", name=f"acc{fq}")
                seen = [0] * NCL
                for s, j, hc in order:
                    for qr in range(4):
                        xt = x_sb[fq][j + hc][:]
                        rhs = bass.AP(
                            tensor=xt.tensor,
                            offset=xt.offset + 33 * (qr & 1) + (qr >> 1),
                            ap=[
                                list(xt.ap[0]),  # partition
                                [66, BFQ],       # b
                                [1, 32],         # nr_x
                            ],
                        )
                        nc.tensor.matmul(
                            ps[32 * j: 32 * j + 32, :],
                            w_sb[:, j, qr, hc, :],
                            rhs,
                            start=(seen[j] == 0),
                            stop=(seen[j] == 7),
                            tile_position=(0, 32 * j),
                        )
                        seen[j] += 1
                        # evac region j as soon as its 8 chunks are in
                        if seen[j] == 8:
                            ev = opool.tile([32, 512], f32, tag="ev",
                                            name=f"ev{fq}_{j}")
                            nc.scalar.activation(
                                out=ev[:], in_=ps[32 * j: 32 * j + 32, :],
                                func=mybir.ActivationFunctionType.Identity,
                                bias=b_sb[32 * j: 32 * j + 32], scale=1.0,
                            )
                            nc.scalar.dma_start(out=out_d[fq, j], in_=ev[:])
    nc.compile()
    return nc


def _prep_inputs(x, weight, bias):
    """Host-side packing into the transposed (mod-8 row, mod-16 col)
    partition layout; bf16 cast.  Returns per-core in_maps."""
    x = np.asarray(x, dtype=np.float32)
    weight = np.asarray(weight, dtype=np.float32)
    bias = np.asarray(bias, dtype=np.float32)

    xpad = np.zeros((B, FULL, FULL), dtype=np.float32)
    xpad[:, PH:PH + H, PW:PW + W] = x[:, 0]
    xpb = xpad.astype(BF16)

    # r = 16*idx + 8*par + dl
    dl = np.arange(8)[:, None, None]
    par = np.arange(2)[None, :, None]
    idx = np.arange(33)[None, None, :]
    r_map = 16 * idx + 8 * par + dl                      # [8, 2, 33]

    w4 = weight.reshape(32, 32, 32, 32)                  # [nr, nc, dr, dc]
    bv = bias.reshape(32, 32)                            # [nr, nc]

    in_maps = []
    for k in range(NCORES):
        c_map = (16 * (4 * k + np.arange(M))[:, None]
                 + np.arange(16)[None, :])               # [m, c16]
        # gather -> [b, dl, par, idx, m, c16]
        g = xpb[:, r_map.reshape(8, 2, 33, 1, 1),
                c_map.reshape(1, 1, 1, M, 16)]
        # -> [fq, bi, dl, par, idx, m, c16]
        g = g.reshape(FQ, BFQ, 8, 2, 33, M, 16)
        # -> [fq, m, dl, c16, bi, par, idx]
        g = g.transpose(0, 5, 2, 6, 1, 3, 4)
        xs = np.ascontiguousarray(g.reshape(FQ, M, 128, BFQ, 2, 33))

        # weights: [nr, j, qr, dl, hc, c16] -> [dl, c16, j, qr, hc, nr]
        wk = w4[:, 4 * k:4 * k + NCL].reshape(32, NCL, 4, 8, 2, 16)
        wk = wk.transpose(3, 5, 1, 2, 4, 0)
        wpk = np.ascontiguousarray(wk.reshape(128, NCL, 4, 2, 32)).astype(BF16)

        # bias: partition 32j + nr_w -> bias[nr_w, 4k+j]
        bk = np.ascontiguousarray(
            bv[:, 4 * k:4 * k + NCL].T.reshape(128, 1))

        in_maps.append({"xs": xs, "wp": wpk, "bp": bk})
    return in_maps


def kernel(x, weight, bias):
    global LAST_RESULTS
    from concourse.bass_utils import run_bass_kernel_spmd

    if "nc" not in _CACHE:
        _CACHE["nc"] = _build_program()
    nc = _CACHE["nc"]

    in_maps = _prep_inputs(x, weight, bias)
    res = run_bass_kernel_spmd(
        nc, in_maps, core_ids=list(range(NCORES)), trace=TRACE
    )
    LAST_RESULTS = res

    out = np.empty((B, NKH, NKW), dtype=np.float32)
    ar = np.arange(32)
    for k in range(NCORES):
        r5 = res.results[k]["out"].reshape(FQ, NCL, 32, BFQ, 32)
        d = r5[:, :, ar, :, ar]                 # [nr, fq, j, bi]
        d = d.transpose(1, 3, 0, 2)             # [fq, bi, nr, j]
        out[:, :, 4 * k:4 * k + NCL] = d.reshape(B, NKH, NCL)
    return out


# revision 11
# speedup vs baseline: 1.7033x; 1.5068x over previous
"""BlockSparseLocallyConnected forward on 8 Trainium2 NeuronCores.

Window-column shard: core k owns output columns nc in {4k..4k+3}, all 64
batches.  The PE does the real MACs (the DVE tensor_tensor path is capped
at 2x = 34us/core; the PE stream floor is 27.3us/core):

  out[b, nr, nc] = sum_{dr,dc} xpad[b, 16nr+dr, 16nc+dc] * w[nr*32+nc, dr*32+dc]

Contraction (dr, dc) is split into 8 chunks q=(qr, hc) of 128 = (dr_local 8,
c16 16); SBUF partition p = 16*dr_local + c16 holds x rows r = dr_local
(mod 8), cols c = c16 (mod 16) -- window columns start at multiples of 16,
so ONE copy of x serves every (nc, hc) with a pure free-dim offset.  Rows
are stored per partition as [b, m', par, idx] with r = 16*idx + 8*par +
dr_local, so the moving AP for window-row nr_x is contiguous (stride 1).

Per (nc_local j, q): lhsT = weights [128, 32 nr_w] (stationary), rhs = x
[128, (b 16, nr_x 32) = 512] (moving), accumulated over the 8 q-chunks into
PSUM[32j:32j+32, 512] via tile_position=(0, 32j).  The matmul computes all
(nr_w, nr_x) cross terms; only the diagonal nr_w == nr_x is the real
output.  ACT evacuates PSUM -> SBUF adding the per-partition bias, the
full [128, 512] tiles DMA out, and the host gathers the diagonal (free).
"""

import sys

sys.path.insert(0, "/opt/trn_rl_repo")

import numpy as np
import ml_dtypes

# ---- problem constants (hardcoded; kernel.py must be self-contained) ----
B = 64            # batch
H = W = 512
PH = PW = 8
FULL = 528        # padded H/W
NKH = NKW = 32    # window grid
NCORES = 8
NCL = 4           # window-columns per core
FQ = 4            # f-dim chunks (16 batches each)
BFQ = B // FQ     # 16
M = 5             # 16-col blocks per core span (80 cols)

BF16 = ml_dtypes.bfloat16

_CACHE = {}

TRACE = False          # test.py sets True to get exec_time_ns
LAST_RESULTS = None    # BassKernelResults of last run (for test.py)


def _build_program():
    import concourse.bass as bass
    import concourse.bacc as bacc
    import concourse.tile as tile
    from concourse import mybir

    dt_c = mybir.dt.bfloat16
    f32 = mybir.dt.float32

    nc = bacc.Bacc(
        "TRN2", target_bir_lowering=False, debug=False, num_devices=NCORES
    )
    # x: [fq, m, p, bi, par, idx] -- each (fq, m) slab is one contiguous DMA
    xs = nc.dram_tensor("xs", [FQ, M, 128, BFQ, 2, 33], dt_c, kind="ExternalInput")
    # weights: [p, j, qr, hc, nr_w]
    wp = nc.dram_tensor("wp", [128, NCL, 4, 2, 32], dt_c, kind="ExternalInput")
    bp = nc.dram_tensor("bp", [128, 1], f32, kind="ExternalInput")
    out_d = nc.dram_tensor("out", [FQ, 128, 512], f32, kind="ExternalOutput")

    with tile.TileContext(nc) as tc:
        with (
            tc.tile_pool(name="xpool", bufs=FQ * M) as xpool,
            tc.tile_pool(name="cst", bufs=1) as cst,
            tc.tile_pool(name="psum", bufs=3, space="PSUM") as psum,
            tc.tile_pool(name="opool", bufs=2 * NCL) as opool,
        ):
            # Tiny wake transfers on each issue ring: DMA engines take ~3us
            # to come up after their first descriptor, so poke them at t=0.
            wk_sb = cst.tile([128, 2], dt_c)
            nc.sync.dma_start(out=wk_sb[:], in_=xs[0, 0, :, 0, 0, 0:2])
            nc.scalar.dma_start(out=wk_sb[:], in_=xs[0, 1, :, 0, 0, 0:2])
            nc.gpsimd.dma_start(out=wk_sb[:], in_=xs[0, 2, :, 0, 0, 0:2])

            # PE warmup during the DMA ramp: back-to-back matmuls push HAM
            # toward full clock before the real stream starts.  (wpsum is
            # never read; the memset runs on the otherwise-idle gpsimd.)
            warm = cst.tile([128, 512], dt_c)
            nc.gpsimd.memset(warm[:], 1.0)
            wpsum = psum.tile([128, 512], f32, tag="warm")
            for _ in range(5):
                nc.tensor.matmul(wpsum[:], warm[:, 0:128], warm[:],
                                 start=True, stop=True)

            # All input DMAs on ONE ring (sync), in exact consumption order:
            # competing rings starve the stream (measured: PE gaps every
            # ~8 MMs when x slabs round-robin over rings).
            w_sb = cst.tile([128, NCL, 4, 2, 32], dt_c)
            nc.sync.dma_start(out=w_sb[:], in_=wp[:])
            b_sb = cst.tile([128, 1], f32)
            nc.gpsimd.dma_start(out=b_sb[:], in_=bp[:])
            x_sb = [[None] * M for _ in range(FQ)]
            for fq in range(FQ):
                for m in range(M):
                    x_sb[fq][m] = xpool.tile(
                        [128, BFQ, 2, 33], dt_c, tag="xb", name=f"xb_{fq}_{m}"
                    )
                    nc.sync.dma_start(out=x_sb[fq][m][:], in_=xs[fq, m])

            # Real stream: per fq, 8 q-chunks x 4 j = 32 matmuls into one
            # PSUM bank.  Ordered by s = j + hc so each matmul only needs
            # x slab m' = j + hc (DMA arrives m-ascending).
            order = []  # (s, j, hc)
            for j in range(NCL):
                order.append((j + 0, j, 0))
                order.append((j + 1, j, 1))
            order.sort()
            for fq in range(FQ):
                ps = psum.tile([128, 512], f32, tag="acc", name=f"acc{fq}")
                seen = [0] * NCL
                for s, j, hc in order:
                    for qr in range(4):
                        xt = x_sb[fq][j + hc][:]
                        rhs = bass.AP(
                            tensor=xt.tensor,
                            offset=xt.offset + 33 * (qr & 1) + (qr >> 1),
                            ap=[
                                list(xt.ap[0]),  # partition
                                [66, BFQ],       # b
                                [1, 32],         # nr_x
                            ],
                        )
                        nc.tensor.matmul(
                            ps[32 * j: 32 * j + 32, :],
                            w_sb[:, j, qr, hc, :],
                            rhs,
                            start=(seen[j] == 0),
                            stop=(seen[j] == 7),
                            tile_position=(0, 32 * j),
                        )
                        seen[j] += 1
                # ONE evac per fq tile: a per-j evac would stall the
                # following same-tile MMs on a write-after-read hazard
                # (measured: ~750ns PE gap per evac + HAM downclock).
                ev = opool.tile([128, 512], f32, tag="ev", name=f"ev{fq}")
                nc.scalar.activation(
                    out=ev[:], in_=ps[:],
                    func=mybir.ActivationFunctionType.Identity,
                    bias=b_sb[:], scale=1.0,
                )
                nc.scalar.dma_start(out=out_d[fq], in_=ev[:])
    nc.compile()
    return nc


def _prep_inputs(x, weight, bias):
    """Host-side packing into the transposed (mod-8 row, mod-16 col)
    partition layout; bf16 cast.  Returns per-core in_maps."""
    x = np.asarray(x, dtype=np.float32)
    weight = np.asarray(weight, dtype=np.float32)
    bias = np.asarray(bias, dtype=np.float32)

    xpad = np.zeros((B, FULL, FULL), dtype=np.float32)
    xpad[:, PH:PH + H, PW:PW + W] = x[:, 0]
    xpb = xpad.astype(BF16)

    # r = 16*idx + 8*par + dl
    dl = np.arange(8)[:, None, None]
    par = np.arange(2)[None, :, None]
    idx = np.arange(33)[None, None, :]
    r_map = 16 * idx + 8 * par + dl                      # [8, 2, 33]

    w4 = weight.reshape(32, 32, 32, 32)                  # [nr, nc, dr, dc]
    bv = bias.reshape(32, 32)                            # [nr, nc]

    in_maps = []
    for k in range(NCORES):
        c_map = (16 * (4 * k + np.arange(M))[:, None]
                 + np.arange(16)[None, :])               # [m, c16]
        # gather -> [b, dl, par, idx, m, c16]
        g = xpb[:, r_map.reshape(8, 2, 33, 1, 1),
                c_map.reshape(1, 1, 1, M, 16)]
        # -> [fq, bi, dl, par, idx, m, c16]
        g = g.reshape(FQ, BFQ, 8, 2, 33, M, 16)
        # -> [fq, m, dl, c16, bi, par, idx]
        g = g.transpose(0, 5, 2, 6, 1, 3, 4)
        xs = np.ascontiguousarray(g.reshape(FQ, M, 128, BFQ, 2, 33))

        # weights: [nr, j, qr, dl, hc, c16] -> [dl, c16, j, qr, hc, nr]
        wk = w4[:, 4 * k:4 * k + NCL].reshape(32, NCL, 4, 8, 2, 16)
        wk = wk.transpose(3, 5, 1, 2, 4, 0)
        wpk = np.ascontiguousarray(wk.reshape(128, NCL, 4, 2, 32)).astype(BF16)

        # bias: partition 32j + nr_w -> bias[nr_w, 4k+j]
        bk = np.ascontiguousarray(
            bv[:, 4 * k:4 * k + NCL].T.reshape(128, 1))

        in_maps.append({"xs": xs, "wp": wpk, "bp": bk})
    return in_maps


def kernel(x, weight, bias):
    global LAST_RESULTS
    from concourse.bass_utils import run_bass_kernel_spmd

    if "nc" not in _CACHE:
        _CACHE["nc"] = _build_program()
    nc = _CACHE["nc"]

    in_maps = _prep_inputs(x, weight, bias)
    res = run_bass_kernel_spmd(
        nc, in_maps, core_ids=list(range(NCORES)), trace=TRACE
    )
    LAST_RESULTS = res

    out = np.empty((B, NKH, NKW), dtype=np.float32)
    ar = np.arange(32)
    for k in range(NCORES):
        r5 = res.results[k]["out"].reshape(FQ, NCL, 32, BFQ, 32)
        d = r5[:, :, ar, :, ar]                 # [nr, fq, j, bi]
        d = d.transpose(1, 3, 0, 2)             # [fq, bi, nr, j]
        out[:, :, 4 * k:4 * k + NCL] = d.reshape(B, NKH, NCL)
    return out


# revision 13
# speedup vs baseline: 1.7153x; 1.0070x over previous
"""BlockSparseLocallyConnected forward on 8 Trainium2 NeuronCores.

Window-column shard: core k owns output columns nc in {4k..4k+3}, all 64
batches.  The PE does the real MACs (the DVE tensor_tensor path is capped
at 2x = 34us/core; the PE stream floor is 27.3us/core):

  out[b, nr, nc] = sum_{dr,dc} xpad[b, 16nr+dr, 16nc+dc] * w[nr*32+nc, dr*32+dc]

Contraction (dr, dc) is split into 8 chunks q=(qr, hc) of 128 = (dr_local 8,
c16 16); SBUF partition p = 16*dr_local + c16 holds x rows r = dr_local
(mod 8), cols c = c16 (mod 16) -- window columns start at multiples of 16,
so ONE copy of x serves every (nc, hc) with a pure free-dim offset.  Rows
are stored per partition as [b, m', par, idx] with r = 16*idx + 8*par +
dr_local, so the moving AP for window-row nr_x is contiguous (stride 1).

Per (nc_local j, q): lhsT = weights [128, 32 nr_w] (stationary), rhs = x
[128, (b 16, nr_x 32) = 512] (moving), accumulated over the 8 q-chunks into
PSUM[32j:32j+32, 512] via tile_position=(0, 32j).  The matmul computes all
(nr_w, nr_x) cross terms; only the diagonal nr_w == nr_x is the real
output.  ACT evacuates PSUM -> SBUF adding the per-partition bias, the
full [128, 512] tiles DMA out, and the host gathers the diagonal (free).
"""

import sys

sys.path.insert(0, "/opt/trn_rl_repo")

import numpy as np
import ml_dtypes

# ---- problem constants (hardcoded; kernel.py must be self-contained) ----
B = 64            # batch
H = W = 512
PH = PW = 8
FULL = 528        # padded H/W
NKH = NKW = 32    # window grid
NCORES = 8
NCL = 4           # window-columns per core
FQ = 4            # f-dim chunks (16 batches each)
BFQ = B // FQ     # 16
M = 5             # 16-col blocks per core span (80 cols)

BF16 = ml_dtypes.bfloat16

_CACHE = {}

TRACE = False          # test.py sets True to get exec_time_ns
LAST_RESULTS = None    # BassKernelResults of last run (for test.py)


def _build_program():
    import concourse.bass as bass
    import concourse.bacc as bacc
    import concourse.tile as tile
    from concourse import mybir

    dt_c = mybir.dt.bfloat16
    f32 = mybir.dt.float32

    nc = bacc.Bacc(
        "TRN2", target_bir_lowering=False, debug=False, num_devices=NCORES
    )
    # x: [fq, m, p, bi, par, idx] -- each (fq, m) slab is one contiguous DMA
    xs = nc.dram_tensor("xs", [FQ, M, 128, BFQ, 2, 33], dt_c, kind="ExternalInput")
    # weights: [p, j, qr, hc, nr_w]
    wp = nc.dram_tensor("wp", [128, NCL, 4, 2, 32], dt_c, kind="ExternalInput")
    bp = nc.dram_tensor("bp", [128, 1], f32, kind="ExternalInput")
    out_d = nc.dram_tensor("out", [FQ, 128, 512], f32, kind="ExternalOutput")

    with tile.TileContext(nc) as tc:
        with (
            tc.tile_pool(name="xpool", bufs=FQ * M) as xpool,
            tc.tile_pool(name="cst", bufs=1) as cst,
            tc.tile_pool(name="psum", bufs=2, space="PSUM") as psum,
            tc.tile_pool(name="opool", bufs=NCL) as opool,
        ):
            # All input DMAs on ONE ring (sync), in exact consumption order:
            # competing rings starve the stream (measured: PE gaps every
            # ~8 MMs when x slabs round-robin over rings).  No PE warmup:
            # the PE queue is strict FIFO, so warmup matmuls only push the
            # real stream back -- let the first real MMs ramp HAM instead.
            w_sb = [None] * NCL
            for j in range(NCL):
                w_sb[j] = cst.tile([128, 4, 2, 32], dt_c, name=f"w{j}")
            b_sb = cst.tile([128, 1], f32)
            x_sb = [[None] * M for _ in range(FQ)]
            for fq in range(FQ):
                for m in range(M):
                    x_sb[fq][m] = xpool.tile(
                        [128, BFQ, 2, 33], dt_c, tag="xb", name=f"xb_{fq}_{m}"
                    )
            # issue order = consumption order of the s-sorted MM stream
            nc.sync.dma_start(out=w_sb[0][:], in_=wp[:, 0])
            nc.sync.dma_start(out=x_sb[0][0][:], in_=xs[0, 0])
            nc.sync.dma_start(out=x_sb[0][1][:], in_=xs[0, 1])
            nc.sync.dma_start(out=w_sb[1][:], in_=wp[:, 1])
            nc.sync.dma_start(out=x_sb[0][2][:], in_=xs[0, 2])
            nc.sync.dma_start(out=w_sb[2][:], in_=wp[:, 2])
            nc.sync.dma_start(out=x_sb[0][3][:], in_=xs[0, 3])
            nc.sync.dma_start(out=w_sb[3][:], in_=wp[:, 3])
            nc.sync.dma_start(out=x_sb[0][4][:], in_=xs[0, 4])
            nc.sync.dma_start(out=b_sb[:], in_=bp[:])
            for fq in range(1, FQ):
                for m in range(M):
                    nc.sync.dma_start(out=x_sb[fq][m][:], in_=xs[fq, m])

            # Real stream: per fq, 8 q-chunks x 4 j = 32 matmuls into one
            # PSUM bank.  Ordered by s = j + hc so each matmul only needs
            # x slab m' = j + hc (DMA arrives m-ascending).
            order = []  # (s, j, hc)
            for j in range(NCL):
                order.append((j + 0, j, 0))
                order.append((j + 1, j, 1))
            order.sort()
            for fq in range(FQ):
                ps = psum.tile([128, 512], f32, tag="acc", name=f"acc{fq}")
                seen = [0] * NCL
                for s, j, hc in order:
                    for qr in range(4):
                        xt = x_sb[fq][j + hc][:]
                        rhs = bass.AP(
                            tensor=xt.tensor,
                            offset=xt.offset + 33 * (qr & 1) + (qr >> 1),
                            ap=[
                                list(xt.ap[0]),  # partition
                                [66, BFQ],       # b
                                [1, 32],         # nr_x
                            ],
                        )
                        nc.tensor.matmul(
                            ps[32 * j: 32 * j + 32, :],
                            w_sb[j][:, qr, hc, :],
                            rhs,
                            start=(seen[j] == 0),
                            stop=(seen[j] == 7),
                            tile_position=(0, 32 * j),
                        )
                        seen[j] += 1
                # ONE evac per fq tile: a per-j evac would stall the
                # following same-tile MMs on a write-after-read hazard
                # (measured: ~750ns PE gap per evac + HAM downclock).
                ev = opool.tile([128, 512], f32, tag="ev", name=f"ev{fq}")
                nc.scalar.activation(
                    out=ev[:], in_=ps[:],
                    func=mybir.ActivationFunctionType.Identity,
                    bias=b_sb[:], scale=1.0,
                )
                nc.scalar.dma_start(out=out_d[fq], in_=ev[:])
    nc.compile()
    return nc


def _prep_inputs(x, weight, bias):
    """Host-side packing into the transposed (mod-8 row, mod-16 col)
    partition layout; bf16 cast.  Returns per-core in_maps."""
    x = np.asarray(x, dtype=np.float32)
    weight = np.asarray(weight, dtype=np.float32)
    bias = np.asarray(bias, dtype=np.float32)

    xpad = np.zeros((B, FULL, FULL), dtype=np.float32)
    xpad[:, PH:PH + H, PW:PW + W] = x[:, 0]
    xpb = xpad.astype(BF16)

    # r = 16*idx + 8*par + dl
    dl = np.arange(8)[:, None, None]
    par = np.arange(2)[None, :, None]
    idx = np.arange(33)[None, None, :]
    r_map = 16 * idx + 8 * par + dl                      # [8, 2, 33]

    w4 = weight.reshape(32, 32, 32, 32)                  # [nr, nc, dr, dc]
    bv = bias.reshape(32, 32)                            # [nr, nc]

    in_maps = []
    for k in range(NCORES):
        c_map = (16 * (4 * k + np.arange(M))[:, None]
                 + np.arange(16)[None, :])               # [m, c16]
        # gather -> [b, dl, par, idx, m, c16]
        g = xpb[:, r_map.reshape(8, 2, 33, 1, 1),
                c_map.reshape(1, 1, 1, M, 16)]
        # -> [fq, bi, dl, par, idx, m, c16]
        g = g.reshape(FQ, BFQ, 8, 2, 33, M, 16)
        # -> [fq, m, dl, c16, bi, par, idx]
        g = g.transpose(0, 5, 2, 6, 1, 3, 4)
        xs = np.ascontiguousarray(g.reshape(FQ, M, 128, BFQ, 2, 33))

        # weights: [nr, j, qr, dl, hc, c16] -> [dl, c16, j, qr, hc, nr]
        wk = w4[:, 4 * k:4 * k + NCL].reshape(32, NCL, 4, 8, 2, 16)
        wk = wk.transpose(3, 5, 1, 2, 4, 0)
        wpk = np.ascontiguousarray(wk.reshape(128, NCL, 4, 2, 32)).astype(BF16)

        # bias: partition 32j + nr_w -> bias[nr_w, 4k+j]
        bk = np.ascontiguousarray(
            bv[:, 4 * k:4 * k + NCL].T.reshape(128, 1))

        in_maps.append({"xs": xs, "wp": wpk, "bp": bk})
    return in_maps


def kernel(x, weight, bias):
    global LAST_RESULTS
    from concourse.bass_utils import run_bass_kernel_spmd

    if "nc" not in _CACHE:
        _CACHE["nc"] = _build_program()
    nc = _CACHE["nc"]

    in_maps = _prep_inputs(x, weight, bias)
    res = run_bass_kernel_spmd(
        nc, in_maps, core_ids=list(range(NCORES)), trace=TRACE
    )
    LAST_RESULTS = res

    out = np.empty((B, NKH, NKW), dtype=np.float32)
    ar = np.arange(32)
    for k in range(NCORES):
        r5 = res.results[k]["out"].reshape(FQ, NCL, 32, BFQ, 32)
        d = r5[:, :, ar, :, ar]                 # [nr, fq, j, bi]
        d = d.transpose(1, 3, 0, 2)             # [fq, bi, nr, j]
        out[:, :, 4 * k:4 * k + NCL] = d.reshape(B, NKH, NCL)
    return out
